# revision 23
# baseline (speedup 1.0000x reference)
"""DeltaNet-style fast-weight kernel for Trainium2 (8 NeuronCores, data-parallel over batch).

Math (per batch element b):
  h = embed[seq]; x = h + MLP(h); h = LN(x)                      [L=512 tokens, H=64]
  kn_t = h_t/||h_t||; backward scan z_{510}=q=h_511: c_t = kn_t.z_t, z_{t-1}=z_t-c_t kn_t
  y = sum_t c_t h_t = sum_t (c_t s_t) kn_t  with s_t = ||x-mu||*rstd
  out = (y @ rp_w + rp_b) @ out_w + out_b

Scan restructure ("c-space blocked solve"): c solves (I+U)c = Kn q with
U = triu(Kn Kn^T, 1).  Blocks of T=64 steps, processed backward:
  n[t] (block k) = kn_t . z_k                  (PE `gm` matmuls, batch-pair stacked)
  in-block solve: 63 serial DVE ops            n[t] -= n[s] * G[t,s]
  z_{k-1} = z_k - sum_{s in block} c_s kn_s    (PE `z-update` matmuls)
All per-pair matmuls use a (b01,h)-stacked fp16 layout: 2 batches share the
128-wide contract dim, zero-padded halves isolate them.  The serial chain
drops from 511 x ~660ns (two fat DVE ops) to 504 x ~270ns (one small op).

Layouts:
  tokens (phased): partition 4*b + (l%4), chunk l//4  [b = true batch 0..31]
  n rows (sigma):  partition 2*(b%16) + b//16         [pair p = b%16, c01 = b//16]
  stacked fp16:    partition 64*(b//16) + h
"""

import os
import sys

import numpy as np

for _p in ("/opt/trn_rl_repo", "/root/.axon_site/_ro/trn_rl_repo"):
    if os.path.isdir(_p) and _p not in sys.path:
        sys.path.insert(0, _p)

import concourse.bass as bass
import concourse.tile as tile
from concourse import bacc, mybir
from concourse.bass_utils import run_bass_kernel_spmd
from concourse.masks import make_identity

F32 = mybir.dt.float32
F16 = mybir.dt.float16
I32 = mybir.dt.int32
AF = mybir.ActivationFunctionType
OP = mybir.AluOpType

B, L, H, V = 256, 512, 64, 64
NCORES = 8
BS = B // NCORES          # 32 batches per core
LT = 16                   # l-values per tile
NT = L // LT              # 32 tiles
TPT = LT * BS             # 512 tokens per tile
TB = 64                   # scan block size
NB = L // TB              # 8 scan blocks
LN_EPS = 1e-5


def apx(sl, ap_list, off=0):
    """Custom AP anchored at slice `sl` (flat element offsets/strides)."""
    return bass.AP(tensor=sl.tensor, offset=sl.offset + off, ap=ap_list)


def bcf(sl, n):
    """Per-partition scalar broadcast along free dim."""
    return bass.AP(tensor=sl.tensor, offset=sl.offset, ap=[sl.ap[0], [0, n]])


def build_program(ln_trivial: bool):
    nc = bacc.Bacc(None, target_bir_lowering=False)

    seq_p = nc.declare_dram_parameter("seq", [BS, L], I32, isOutput=False)
    embed_p = nc.declare_dram_parameter("embed", [V, H], F32, isOutput=False)
    w1_p = nc.declare_dram_parameter("w1", [H, 2 * H], F32, isOutput=False)
    b1_p = nc.declare_dram_parameter("b1", [2 * H, 1], F32, isOutput=False)
    w2_p = nc.declare_dram_parameter("w2", [2 * H, H], F32, isOutput=False)
    b2_p = nc.declare_dram_parameter("b2", [H, 1], F32, isOutput=False)
    ln_g_p = nc.declare_dram_parameter("ln_g", [1, H], F32, isOutput=False)
    ln_b_p = nc.declare_dram_parameter("ln_b", [1, H], F32, isOutput=False)
    rp_w_p = nc.declare_dram_parameter("rp_w", [H, H], F32, isOutput=False)
    rp_b_p = nc.declare_dram_parameter("rp_b", [H, 1], F32, isOutput=False)
    out_w_p = nc.declare_dram_parameter("out_w", [H, V], F32, isOutput=False)
    out_b_p = nc.declare_dram_parameter("out_b", [V, 1], F32, isOutput=False)
    out_p = nc.declare_dram_parameter("out", [BS, V], F32, isOutput=True)
    if os.environ.get("KDBG", "0") == "1":
        dbg_knph = nc.declare_dram_parameter("dbg_knph", [128, 128 * H], F16, isOutput=True)
        dbg_knd = nc.declare_dram_parameter("dbg_knd", [128, 16 * L], F16, isOutput=True)
        dbg_n = nc.declare_dram_parameter("dbg_n", [BS, L], F32, isOutput=True)
        dbg_gd7 = nc.declare_dram_parameter("dbg_gd7", [BS, TB * TB], F16, isOutput=True)
        dbg_zp = nc.declare_dram_parameter("dbg_zp", [128, BS], F32, isOutput=True)
        dbg_sall = nc.declare_dram_parameter("dbg_sall", [128, 128], F32, isOutput=True)

    # seq staged per tile as 512 f32 in (g-major, 4b+lrel) token order
    seqT_d = nc.dram_tensor("seqT_scratch", [NT, 512], F32)

    from contextlib import ExitStack

    with tile.TileContext(nc) as tc, ExitStack() as ctx:
        consts = ctx.enter_context(tc.tile_pool(name="consts", bufs=1))
        big = ctx.enter_context(tc.tile_pool(name="big", bufs=1))
        work = ctx.enter_context(tc.tile_pool(name="work", bufs=4))
        ps_b = ctx.enter_context(tc.tile_pool(name="ps_b", bufs=1, space="PSUM"))
        ps_x = ctx.enter_context(tc.tile_pool(name="ps_x", bufs=2, space="PSUM"))
        ps_k = ctx.enter_context(tc.tile_pool(name="ps_k", bufs=2, space="PSUM"))
        ps_g = ctx.enter_context(tc.tile_pool(name="ps_g", bufs=1, space="PSUM"))
        ps_t = ctx.enter_context(tc.tile_pool(name="ps_t", bufs=1, space="PSUM"))
        ps_m = ctx.enter_context(tc.tile_pool(name="ps_m", bufs=1, space="PSUM"))

        # ---------------- constants / params ----------------
        ident = consts.tile([H, H], F32)
        make_identity(nc, ident)
        identh = consts.tile([128, 128], F16)
        make_identity(nc, identh)
        identf = consts.tile([128, 128], F32)
        make_identity(nc, identf)

        eps_sb = consts.tile([128, 1], F32)
        nc.vector.memset(eps_sb, LN_EPS)

        viota_i = consts.tile([V, 1], I32)
        nc.gpsimd.iota(viota_i, pattern=[[1, 1]], base=0, channel_multiplier=1)
        viota = consts.tile([V, 1], F32)
        nc.vector.tensor_copy(viota, viota_i)

        embed_sb = consts.tile([V, H], F32)
        w1_sb = consts.tile([H, 2 * H], F32)
        b1_sb = consts.tile([2 * H, 1], F32)
        w2_sb = consts.tile([2 * H, H], F32)
        rp_w_sb = consts.tile([H, H], F32)
        rp_b_sb = consts.tile([H, 1], F32)
        out_w_sb = consts.tile([H, V], F32)
        out_b_sb = consts.tile([V, 1], F32)
        for sb, p in (
            (embed_sb, embed_p), (w1_sb, w1_p), (b1_sb, b1_p), (w2_sb, w2_p),
            (rp_w_sb, rp_w_p), (rp_b_sb, rp_b_p),
            (out_w_sb, out_w_p), (out_b_sb, out_b_p),
        ):
            nc.sync.dma_start(out=sb, in_=p[:, :])

        b2B = consts.tile([128, H], F32)
        nc.sync.dma_start(
            out=b2B,
            in_=bass.AP(tensor=b2_p, offset=0, ap=[[0, 128], [1, H]]),
        )
        if not ln_trivial:
            g_bc = consts.tile([128, H], F32)
            bta_bc = consts.tile([128, H], F32)
            nc.sync.dma_start(
                out=g_bc, in_=bass.AP(tensor=ln_g_p, offset=0, ap=[[0, 128], [1, H]]))
            nc.sync.dma_start(
                out=bta_bc, in_=bass.AP(tensor=ln_b_p, offset=0, ap=[[0, 128], [1, H]]))

        seq_i = consts.tile([BS, L], I32)
        nc.sync.dma_start(out=seq_i, in_=seq_p[:, :])
        seq_f = consts.tile([BS, L], F32)
        nc.vector.tensor_copy(seq_f, seq_i)

        dummy_ps = ps_m.tile([128, 128], F32, tag="psS")
        nc.tensor.matmul(dummy_ps[0:BS, 0:BS], lhsT=ident[0:BS, 0:BS], rhs=ident[0:BS, 0:BS], start=True, stop=True)

        # transpose seq to [l, b] then scatter to phased DRAM staging:
        # seqT_sb[part p=16*ir+4*g+lrel, k, b] -> seqP[i=8k+ir, g*128+4b+lrel]
        seqT_sb = consts.tile([128, 4, BS], F32)
        for k in range(4):
            pst = ps_m.tile([128, 128], F32, tag="psS")
            nc.tensor.matmul(pst[:, 0:BS], lhsT=seq_f[:, 128 * k:128 * (k + 1)], rhs=ident[0:BS, 0:BS], start=True, stop=True)
            nc.vector.tensor_copy(seqT_sb[:, k, :], pst[:, 0:BS])
        # seqQ[b, k, 16ir+4g+lrel] = seqT_sb[16ir+4g+lrel, k, b]
        seqQ = consts.tile([BS, 4, 128], F32)
        for kq in range(4):
            psq = ps_m.tile([128, 128], F32, tag="psS")
            nc.tensor.matmul(psq[0:BS, :], lhsT=apx(seqT_sb[:, :, :], [[128, 128], [1, BS]], off=kq * BS), rhs=identf, start=True, stop=True)
            nc.vector.tensor_copy(seqQ[:, kq, :], psq[0:BS, :])
        # -> seqT_d[i=8k+ir, 128g + 4b + lrel]  (16B contiguous runs)
        for kq in range(4):
            for g in range(4):
                nc.sync.dma_start(
                    out=bass.AP(tensor=seqT_d,
                                offset=kq * 4096 + 128 * g,
                                ap=[[4, BS], [512, 8], [1, 4]]),
                    in_=apx(seqQ[:, :, :], [[512, BS], [16, 8], [1, 4]],
                            off=kq * 128 + 4 * g),
                )

        # embedT, w1p (fp32) then fp16 casts
        pse = ps_m.tile([128, 128], F32, tag="psS")
        nc.tensor.matmul(pse[0:H, 0:H], lhsT=embed_sb, rhs=ident, start=True, stop=True)
        embedT_sb = consts.tile([H, V], F32)
        nc.vector.tensor_copy(embedT_sb, pse[0:H, 0:H])
        psw = ps_m.tile([128, 128], F32, tag="psS")
        nc.tensor.matmul(psw[0:V, :], lhsT=embedT_sb, rhs=w1_sb, start=True, stop=True)
        w1p_sb = consts.tile([V, 2 * H], F32)
        nc.vector.tensor_copy(w1p_sb, psw[0:V, :])

        w1p_h = consts.tile([V, 2 * H], F16)
        embed_h = consts.tile([V, H], F16)
        w2_h = consts.tile([2 * H, H], F16)
        nc.scalar.activation(w1p_h, w1p_sb, AF.Copy)
        nc.scalar.activation(embed_h, embed_sb, AF.Copy)
        nc.scalar.activation(w2_h, w2_sb, AF.Copy)

        # ---------------- persistent buffers ----------------
        kn_ph = big.tile([128, 128, H], F16)     # phased kn  (pstride 8192)
        knd = big.tile([128, 16, L], F16)        # dense stacked kn^T (pstride 8192)
        zpzA = big.tile([128, 16, 128], F16)     # block lhsT (c01,s) (pstride 2048)
        zpzB = big.tile([128, 16, 128], F16)
        nc.gpsimd.memset(zpzA, 0.0)
        nc.gpsimd.memset(zpzB, 0.0)
        GdA = big.tile([BS, TB, TB], F16)        # in-block G (pstride 4096)
        GdB = big.tile([BS, TB, TB], F16)
        ktA = big.tile([H, 16, 128], F16)        # knTok (pstride 2048)
        ktB = big.tile([H, 16, 128], F16)
        GTd = big.tile([128, 16, TB], F16)       # staging (pstride 1024)

        n_sb = big.tile([BS, L], F32)            # sigma rows (pstride 512)
        s_all = big.tile([128, 128], F32)
        c_rep = big.tile([128, 128], F32)
        w_all = big.tile([128, 128], F32)
        y4 = big.tile([128, H], F32)
        nc.vector.memset(y4, 0.0)
        zpadf = big.tile([128, BS], F32)         # pair-major cols (pstride 32)
        nc.vector.memset(zpadf, 0.0)
        zpad16 = big.tile([128, BS], F16)
        nT16 = big.tile([TB, BS], F16)
        gmT16 = big.tile([TB, BS], F16)
        qv = big.tile([BS, H], F16)
        s511 = big.tile([BS, 1], F32)
        qvf = big.tile([BS, H], F32)
        qv16 = big.tile([BS, H], F16)
        qT = big.tile([H, BS], F32)

        zpz = (zpzA, zpzB)
        Gd = (GdA, GdB)
        kt = (ktA, ktB)

        def emit_tile(i):
            seqb = work.tile([V, TPT], F32)
            nc.sync.dma_start(
                out=seqb,
                in_=bass.AP(tensor=seqT_d, offset=i * 512, ap=[[0, V], [1, TPT]]),
            )
            oh = work.tile([V, TPT], F16)
            nc.vector.tensor_scalar(
                out=oh, in0=seqb, scalar1=viota[:, 0:1], scalar2=None,
                op0=OP.is_equal,
            )

            psB = ps_b.tile([2 * H, TPT], F32, tag="psB")
            nc.tensor.matmul(psB, lhsT=w1p_h, rhs=oh, start=True, stop=True)
            rT = work.tile([2 * H, TPT], F16)
            nc.scalar.activation(rT, psB, AF.Relu, bias=b1_sb[:, 0:1])

            psX = ps_x.tile([128, 4, H], F32, tag="psX")
            for g in range(4):
                nc.tensor.matmul(psX[:, g, :], lhsT=oh[:, 128 * g:128 * (g + 1)], rhs=embed_h, start=True, stop=False)
                nc.tensor.matmul(psX[:, g, :], lhsT=rT[:, 128 * g:128 * (g + 1)], rhs=w2_h, start=False, stop=True)

            x_sb = work.tile([128, 4, H], F32)
            nc.vector.tensor_tensor(
                x_sb, psX, apx(b2B[:, :], [[H, 128], [0, 4], [1, H]]), OP.add,
            )

            st = work.tile([128, 4, 6], F32)
            mv = work.tile([128, 4, 2], F32)
            for g in range(4):
                nc.vector.bn_stats(st[:, g, :], x_sb[:, g, :])
            for g in range(4):
                nc.vector.bn_aggr(mv[:, g, :], st[:, g, :])

            nrm = work.tile([128, 4, 1], F32)
            sstd = work.tile([128, 4, 1], F32)
            invn = work.tile([128, 4, 1], F32)
            rstd = work.tile([128, 4, 1], F32)
            var_ap = mv[:, :, 1:2]
            nc.scalar.activation(nrm, var_ap, AF.Sqrt, scale=float(H))
            nc.scalar.activation(sstd, var_ap, AF.Sqrt, bias=eps_sb[:, 0:1])
            nc.vector.reciprocal(invn, nrm)
            nc.vector.reciprocal(rstd, sstd)
            nc.vector.tensor_tensor(
                s_all[:, 4 * i:4 * (i + 1)], nrm[:, :, 0], rstd[:, :, 0], OP.mult,
            )

            kn_t = work.tile([128, 4, H], F32)
            if ln_trivial:
                for g in range(4):
                    nc.vector.tensor_scalar(
                        out=kn_t[:, g, :], in0=x_sb[:, g, :],
                        scalar1=mv[:, g, 0:1], scalar2=invn[:, g, :],
                        op0=OP.subtract, op1=OP.mult,
                    )
            else:
                h_t = work.tile([128, 4, H], F32)
                for g in range(4):
                    mu = mv[:, g, 0:1]
                    nc.vector.tensor_scalar(
                        out=h_t[:, g, :], in0=x_sb[:, g, :],
                        scalar1=mu, scalar2=rstd[:, g, :],
                        op0=OP.subtract, op1=OP.mult,
                    )
                    nc.vector.tensor_mul(h_t[:, g, :], h_t[:, g, :], g_bc)
                    nc.vector.tensor_add(h_t[:, g, :], h_t[:, g, :], bta_bc)
                ss = work.tile([128, 4, 1], F32)
                sn = work.tile([128, 4, 1], F32)
                rn = work.tile([128, 4, 1], F32)
                for g in range(4):
                    nc.vector.scalar_tensor_tensor(
                        out=kn_t[:, g, :], in0=h_t[:, g, :], scalar=1.0,
                        in1=h_t[:, g, :], op0=OP.mult, op1=OP.mult,
                        accum_out=ss[:, g, :],
                    )
                nc.scalar.activation(sn, ss, AF.Sqrt)
                nc.vector.tensor_scalar(sn, sn, 1e-12, None, op0=OP.max)
                nc.vector.reciprocal(rn, sn)
                for g in range(4):
                    nc.vector.tensor_scalar(
                        out=kn_t[:, g, :], in0=h_t[:, g, :],
                        scalar1=rn[:, g, :], scalar2=None, op0=OP.mult,
                    )
                nc.scalar.activation(s_all[:, 4 * i:4 * (i + 1)], sn[:, :, 0], AF.Copy)

            nc.scalar.activation(kn_ph[:, 4 * i:4 * (i + 1), :], kn_t, AF.Copy)

            # stacked fp16 transposes; upper batches land at psum parts 64:128
            psKn = ps_k.tile([128, 4, 128], F32, tag="psKn")
            for g in range(4):
                nc.tensor.matmul(
                    psKn[0:64, g, 0:64],
                    lhsT=kn_ph[0:64, 4 * i + g, :], rhs=identh[0:64, 0:64],
                    start=True, stop=True,
                )
                nc.tensor.matmul(
                    psKn[64:128, g, 64:128],
                    lhsT=kn_ph[64:128, 4 * i + g, :], rhs=identh[64:128, 64:128],
                    start=True, stop=True,
                )
            # evac into knd[(b01,h), p, t] and zpz_blk[(b01,h), p, c01*64+s]
            k = i // 4
            sib0 = (i % 4) * LT
            zt = zpz[k % 2]
            for half in range(2):
                # psKn pstride 512; free (g, col=4b+lrel): g:128, b:4, lrel:1
                src = apx(psKn[:, :, :], [[512, 64], [128, 4], [4, 16], [1, 4]],
                          off=half * (64 * 512 + 64))
                # knd pstride 8192; (p, t=16i+4g+lrel): b->512, g->4, lrel->1
                dk = apx(knd[:, :, :], [[8192, 64], [4, 4], [512, 16], [1, 4]],
                         off=half * 64 * 8192 + 16 * i)
                nc.scalar.activation(dk, src, AF.Copy)
                # zpz pstride 2048; (p, c01*64 + sib0+4g+lrel): b->128, g->4
                dz = apx(zt[:, :, :], [[2048, 64], [4, 4], [128, 16], [1, 4]],
                         off=half * (64 * 2048 + 64) + sib0)
                nc.scalar.activation(dz, src, AF.Copy)

        def emit_block_prep(k):
            zt = zpz[k % 2]
            ktk = kt[k % 2]
            gdk = Gd[k % 2]
            t0 = TB * k
            for p in range(16):
                psG = ps_g.tile([128, TB], F32, tag="psG")
                nc.tensor.matmul(
                    psG, lhsT=zt[:, p, :], rhs=knd[:, p, t0:t0 + TB],
                    start=True, stop=True,
                )
                nc.scalar.activation(GTd[:, p, :], psG, AF.Copy)
                psTk = ps_t.tile([TB, 128], F32, tag="psTk")
                nc.tensor.matmul(
                    psTk, lhsT=knd[:, p, t0:t0 + TB], rhs=identh,
                    start=True, stop=True,
                )
                nc.scalar.activation(ktk[:, p, :], psTk, AF.Copy)
                # Gd[2p+c01, j, i] <- GTd[64*c01+j, p, i]
                nc.sync.dma_start(
                    out=apx(gdk[:, :, :], [[4096, 2], [TB, TB], [1, TB]],
                            off=2 * p * 4096),
                    in_=apx(GTd[:, p, :], [[1024, 128], [1, TB]]),
                )

        def emit_block_scan(k):
            gdk = Gd[k % 2]
            t0 = TB * k
            nc.scalar.activation(zpad16, zpadf, AF.Copy)
            # gm oriented [t, sigma-col] (psum out base must be 0/32/64),
            # then PE-transposed back into n's [sigma, t] layout
            psGmT_f = ps_m.tile([128, 128], F32, tag="psS")
            psGmT = psGmT_f[0:TB, 0:BS]
            for p in range(16):
                nc.tensor.matmul(
                    psGmT[:, 2 * p:2 * p + 2],
                    lhsT=knd[:, p, t0:t0 + TB],
                    rhs=zpad16[:, 2 * p:2 * p + 2],
                    start=True, stop=True,
                )
            nc.scalar.activation(gmT16, psGmT, AF.Copy)
            psN_f = ps_m.tile([128, 128], F32, tag="psS")
            nc.tensor.matmul(psN_f[0:BS, 0:TB], lhsT=gmT16, rhs=identh[0:TB, 0:TB], start=True, stop=True)
            nc.vector.tensor_copy(n_sb[:, t0:t0 + TB], psN_f[0:BS, 0:TB])
            if k == NB - 1:
                nc.vector.memset(n_sb[:, L - 1:L], 0.0)

            # serial in-block solve: n[t] -= n[s] * G[t,s]
            for j in range(TB - 1, 0, -1):
                s = t0 + j
                nc.vector.add_instruction(
                    mybir.InstTensorScalarPtr(
                        name=nc.vector.bass.get_next_instruction_name(),
                        is_scalar_tensor_tensor=True,
                        op0=OP.mult, op1=OP.subtract, reverse1=True,
                        ins=[
                            nc.vector.lower_ap(gdk[:, j, 0:j]),
                            nc.vector.lower_ap(n_sb[:, s:s + 1]),
                            nc.vector.lower_ap(n_sb[:, t0:t0 + j]),
                        ],
                        outs=[nc.vector.lower_ap(n_sb[:, t0:t0 + j])],
                    )
                )

            # y-accum for this block: c_rep (phased) <- n (sigma rows)
            for lrel in range(4):
                nc.sync.dma_start(
                    out=apx(c_rep[:, :], [[512, 16], [1, 16]],
                            off=lrel * 128 + 16 * k),
                    in_=apx(n_sb[:, :], [[1024, 16], [4, 16]], off=t0 + lrel),
                )
                nc.sync.dma_start(
                    out=apx(c_rep[:, :], [[512, 16], [1, 16]],
                            off=64 * 128 + lrel * 128 + 16 * k),
                    in_=apx(n_sb[:, :], [[1024, 16], [4, 16]], off=512 + t0 + lrel),
                )
            nc.vector.tensor_tensor(
                w_all[:, 16 * k:16 * (k + 1)],
                c_rep[:, 16 * k:16 * (k + 1)],
                s_all[:, 16 * k:16 * (k + 1)], OP.mult,
            )
            for ch in range(16 * k + 15, 16 * k - 1, -1):
                nc.vector.scalar_tensor_tensor(
                    out=y4, in0=kn_ph[:, ch, :], scalar=w_all[:, ch:ch + 1],
                    in1=y4, op0=OP.mult, op1=OP.add,
                )

            if k > 0:
                # z-update: z -= sum_{s in block} c_s kn_s
                psT_f = ps_m.tile([128, 128], F32, tag="psS")
                nc.tensor.matmul(psT_f[0:TB, 0:BS], lhsT=n_sb[:, t0:t0 + TB], rhs=ident[0:BS, 0:BS], start=True, stop=True)
                nc.scalar.activation(nT16, psT_f[0:TB, 0:BS], AF.Copy)
                psZ_f = ps_m.tile([128, 128], F32, tag="psS")
                psZ = psZ_f[:, 0:BS]
                ktk = kt[k % 2]
                for p in range(16):
                    nc.tensor.matmul(
                        psZ[:, 2 * p:2 * p + 2],
                        lhsT=ktk[:, p, :], rhs=nT16[:, 2 * p:2 * p + 2],
                        start=True, stop=True,
                    )
                nc.vector.tensor_tensor(
                    apx(zpadf[:, :], [[32, 64], [2, 16]]),
                    apx(zpadf[:, :], [[32, 64], [2, 16]]),
                    apx(psZ_f[:, :], [[128, 64], [2, 16]]),
                    OP.subtract,
                )
                nc.vector.tensor_tensor(
                    apx(zpadf[:, :], [[32, 64], [2, 16]], off=64 * 32 + 1),
                    apx(zpadf[:, :], [[32, 64], [2, 16]], off=64 * 32 + 1),
                    apx(psZ_f[:, :], [[128, 64], [2, 16]], off=64 * 128 + 1),
                    OP.subtract,
                )

        # ---------------- main pipeline, reverse tile order ----------------
        for i in range(NT - 1, -1, -1):
            emit_tile(i)
            if i % 4 == 0:
                k = i // 4
                emit_block_prep(k)
                if k == NB - 1:
                    # q = s_511 * kn_511; seed zpad (pair-major, b01-padded)
                    nc.sync.dma_start(
                        out=qv,
                        in_=apx(kn_ph[:, :, :], [[4 * 8192, BS], [1, H]],
                                off=3 * 8192 + 127 * H),
                    )
                    nc.sync.dma_start(
                        out=s511,
                        in_=apx(s_all[:, :], [[4 * 128, BS], [1, 1]],
                                off=3 * 128 + 127),
                    )
                    nc.vector.tensor_scalar(
                        out=qvf, in0=qv, scalar1=s511[:, 0:1], scalar2=None, op0=OP.mult,
                    )
                    nc.scalar.activation(qv16, qvf, AF.Copy)
                    psQ_f = ps_m.tile([128, 128], F32, tag="psS")
                    nc.tensor.matmul(psQ_f[0:H, 0:BS], lhsT=qv16, rhs=identh[0:BS, 0:BS], start=True, stop=True)
                    nc.scalar.activation(qT, psQ_f[0:H, 0:BS], AF.Copy)
                    nc.vector.tensor_copy(
                        apx(zpadf[:, :], [[32, 64], [2, 16]]),
                        qT[0:64, 0:16],
                    )
                    nc.sync.dma_start(
                        out=apx(zpadf[:, :], [[32, 64], [2, 16]], off=64 * 32 + 1),
                        in_=qT[0:64, 16:BS],
                    )
                emit_block_scan(k)

        if os.environ.get("KDBG", "0") == "1":
            nc.sync.dma_start(out=dbg_knph[:, :], in_=apx(kn_ph[:, :, :], [[8192, 128], [1, 128 * H]]))
            nc.sync.dma_start(out=dbg_knd[:, :], in_=apx(knd[:, :, :], [[8192, 128], [1, 16 * L]]))
            nc.sync.dma_start(out=dbg_n[:, :], in_=n_sb)
            nc.sync.dma_start(out=dbg_gd7[:, :], in_=apx(Gd[(NB - 1) % 2][:, :, :], [[4096, BS], [1, TB * TB]]))
            nc.sync.dma_start(out=dbg_zp[:, :], in_=zpadf)
            nc.sync.dma_start(out=dbg_sall[:, :], in_=s_all)

        # ---------------- tail: y reduce + projections ----------------
        yp = [big.tile([BS, H], F32, name=f"yp{_l}") for _l in range(4)]
        for lrel in range(4):
            nc.sync.dma_start(
                out=yp[lrel],
                in_=apx(y4[:, :], [[4 * H, BS], [1, H]], off=lrel * H),
            )
        y_sb = big.tile([BS, H], F32)
        nc.vector.tensor_add(y_sb, yp[0], yp[1])
        nc.vector.tensor_add(y_sb, y_sb, yp[2])
        nc.vector.tensor_add(y_sb, y_sb, yp[3])

        psF = ps_m.tile([128, 128], F32, tag="psS")
        nc.tensor.matmul(psF[0:H, 0:BS], lhsT=y_sb, rhs=ident[0:BS, 0:BS], start=True, stop=True)
        yT = big.tile([H, BS], F32)
        nc.vector.tensor_copy(yT, psF[0:H, 0:BS])

        psG2 = ps_m.tile([128, 128], F32, tag="psS")
        nc.tensor.matmul(psG2[0:H, 0:BS], lhsT=rp_w_sb, rhs=yT, start=True, stop=True)
        r1 = big.tile([H, BS], F32)
        nc.scalar.activation(r1, psG2[0:H, 0:BS], AF.Identity, bias=rp_b_sb[:, 0:1])

        psH = ps_m.tile([128, 128], F32, tag="psS")
        nc.tensor.matmul(psH[0:V, 0:BS], lhsT=out_w_sb, rhs=r1, start=True, stop=True)
        r2 = big.tile([V, BS], F32)
        nc.scalar.activation(r2, psH[0:V, 0:BS], AF.Identity, bias=out_b_sb[:, 0:1])

        psI = ps_m.tile([128, 128], F32, tag="psS")
        nc.tensor.matmul(psI[0:BS, 0:V], lhsT=r2, rhs=ident, start=True, stop=True)
        o_sb = big.tile([BS, V], F32)
        nc.vector.tensor_copy(o_sb, psI[0:BS, 0:V])
        nc.sync.dma_start(out=out_p[:, :], in_=o_sb)

    nc.finalize()
    return nc


_CACHE = {}


def _run(inputs, trace=False, **kw):
    seq = np.asarray(inputs["seq"]).astype(np.int32)
    embed = np.asarray(inputs["embed"], np.float32)
    w1 = np.asarray(inputs["w1"], np.float32)
    b1 = np.asarray(inputs["b1"], np.float32).reshape(2 * H, 1)
    w2 = np.asarray(inputs["w2"], np.float32)
    b2 = np.asarray(inputs["b2"], np.float32).reshape(H, 1)
    ln_g = np.asarray(inputs["ln_g"], np.float32).reshape(1, H)
    ln_b = np.asarray(inputs["ln_b"], np.float32).reshape(1, H)
    rp_w = np.asarray(inputs["rp_w"], np.float32)
    rp_b = np.asarray(inputs["rp_b"], np.float32).reshape(H, 1)
    out_w = np.asarray(inputs["out_w"], np.float32)
    out_b = np.asarray(inputs["out_b"], np.float32).reshape(V, 1)

    ln_trivial = bool(np.all(ln_g == 1.0) and np.all(ln_b == 0.0))
    if ln_trivial not in _CACHE:
        _CACHE[ln_trivial] = build_program(ln_trivial)
    nc = _CACHE[ln_trivial]

    in_maps = []
    for c in range(NCORES):
        in_maps.append({
            "seq": seq[BS * c:BS * (c + 1)],
            "embed": embed, "w1": w1, "b1": b1, "w2": w2, "b2": b2,
            "ln_g": ln_g, "ln_b": ln_b,
            "rp_w": rp_w, "rp_b": rp_b, "out_w": out_w, "out_b": out_b,
        })
    br = run_bass_kernel_spmd(nc, in_maps, list(range(NCORES)), trace=trace, **kw)
    out = np.concatenate([r["out"] for r in br.results], axis=0)
    return out, br


def kernel(**inputs) -> np.ndarray:
    return _run(inputs)[0]


# revision 25
# speedup vs baseline: 1.0610x; 1.0610x over previous
"""DeltaNet-style fast-weight kernel for Trainium2 (8 NeuronCores, data-parallel over batch).

Math (per batch element b):
  h = embed[seq]; x = h + MLP(h); h = LN(x)                      [L=512 tokens, H=64]
  kn_t = h_t/||h_t||; backward scan z_{510}=q=h_511: c_t = kn_t.z_t, z_{t-1}=z_t-c_t kn_t
  y = sum_t c_t h_t = sum_t (c_t s_t) kn_t  with s_t = ||x-mu||*rstd
  out = (y @ rp_w + rp_b) @ out_w + out_b

Scan restructure ("c-space blocked solve"): c solves (I+U)c = Kn q with
U = triu(Kn Kn^T, 1).  Blocks of T=64 steps, processed backward:
  n[t] (block k) = kn_t . z_k                  (PE `gm` matmuls, batch-pair stacked)
  in-block solve: 63 serial DVE ops            n[t] -= n[s] * G[t,s]
  z_{k-1} = z_k - sum_{s in block} c_s kn_s    (PE `z-update` matmuls)
All per-pair matmuls use a (b01,h)-stacked fp16 layout: 2 batches share the
128-wide contract dim, zero-padded halves isolate them.  The serial chain
drops from 511 x ~660ns (two fat DVE ops) to 504 x ~270ns (one small op).

Layouts:
  tokens (phased): partition 4*b + (l%4), chunk l//4  [b = true batch 0..31]
  n rows (sigma):  partition 2*(b%16) + b//16         [pair p = b%16, c01 = b//16]
  stacked fp16:    partition 64*(b//16) + h
"""

import os
import sys

import numpy as np

for _p in ("/opt/trn_rl_repo", "/root/.axon_site/_ro/trn_rl_repo"):
    if os.path.isdir(_p) and _p not in sys.path:
        sys.path.insert(0, _p)

import concourse.bass as bass
import concourse.tile as tile
from concourse import bacc, mybir
from concourse.bass_utils import run_bass_kernel_spmd
from concourse.masks import make_identity

F32 = mybir.dt.float32
F16 = mybir.dt.float16
I32 = mybir.dt.int32
AF = mybir.ActivationFunctionType
OP = mybir.AluOpType

B, L, H, V = 256, 512, 64, 64
NCORES = 8
BS = B // NCORES          # 32 batches per core
LT = 16                   # l-values per tile
NT = L // LT              # 32 tiles
TPT = LT * BS             # 512 tokens per tile
TB = 64                   # scan block size
NB = L // TB              # 8 scan blocks
LN_EPS = 1e-5


def apx(sl, ap_list, off=0):
    """Custom AP anchored at slice `sl` (flat element offsets/strides)."""
    return bass.AP(tensor=sl.tensor, offset=sl.offset + off, ap=ap_list)


def bcf(sl, n):
    """Per-partition scalar broadcast along free dim."""
    return bass.AP(tensor=sl.tensor, offset=sl.offset, ap=[sl.ap[0], [0, n]])


def build_program(ln_trivial: bool):
    nc = bacc.Bacc(None, target_bir_lowering=False)

    seq_p = nc.declare_dram_parameter("seq", [BS, L], I32, isOutput=False)
    embed_p = nc.declare_dram_parameter("embed", [V, H], F32, isOutput=False)
    w1_p = nc.declare_dram_parameter("w1", [H, 2 * H], F32, isOutput=False)
    b1_p = nc.declare_dram_parameter("b1", [2 * H, 1], F32, isOutput=False)
    w2_p = nc.declare_dram_parameter("w2", [2 * H, H], F32, isOutput=False)
    b2_p = nc.declare_dram_parameter("b2", [H, 1], F32, isOutput=False)
    ln_g_p = nc.declare_dram_parameter("ln_g", [1, H], F32, isOutput=False)
    ln_b_p = nc.declare_dram_parameter("ln_b", [1, H], F32, isOutput=False)
    rp_w_p = nc.declare_dram_parameter("rp_w", [H, H], F32, isOutput=False)
    rp_b_p = nc.declare_dram_parameter("rp_b", [H, 1], F32, isOutput=False)
    out_w_p = nc.declare_dram_parameter("out_w", [H, V], F32, isOutput=False)
    out_b_p = nc.declare_dram_parameter("out_b", [V, 1], F32, isOutput=False)
    out_p = nc.declare_dram_parameter("out", [BS, V], F32, isOutput=True)
    if os.environ.get("KDBG", "0") == "1":
        dbg_knph = nc.declare_dram_parameter("dbg_knph", [128, 128 * H], F16, isOutput=True)
        dbg_knd = nc.declare_dram_parameter("dbg_knd", [128, 16 * L], F16, isOutput=True)
        dbg_n = nc.declare_dram_parameter("dbg_n", [BS, L], F32, isOutput=True)
        dbg_gd7 = nc.declare_dram_parameter("dbg_gd7", [BS, TB * TB], F16, isOutput=True)
        dbg_zp = nc.declare_dram_parameter("dbg_zp", [128, BS], F32, isOutput=True)
        dbg_sall = nc.declare_dram_parameter("dbg_sall", [128, 128], F32, isOutput=True)

    # seq staged per tile as 512 f32 in (g-major, 4b+lrel) token order
    seqT_d = nc.dram_tensor("seqT_scratch", [NT, 512], F32)

    from contextlib import ExitStack

    with tile.TileContext(nc) as tc, ExitStack() as ctx:
        consts = ctx.enter_context(tc.tile_pool(name="consts", bufs=1))
        big = ctx.enter_context(tc.tile_pool(name="big", bufs=1))
        work = ctx.enter_context(tc.tile_pool(name="work", bufs=2))
        ps_b = ctx.enter_context(tc.tile_pool(name="ps_b", bufs=1, space="PSUM"))
        ps_x = ctx.enter_context(tc.tile_pool(name="ps_x", bufs=2, space="PSUM"))
        ps_k = ctx.enter_context(tc.tile_pool(name="ps_k", bufs=2, space="PSUM"))
        ps_g = ctx.enter_context(tc.tile_pool(name="ps_g", bufs=1, space="PSUM"))
        ps_t = ctx.enter_context(tc.tile_pool(name="ps_t", bufs=1, space="PSUM"))
        ps_m = ctx.enter_context(tc.tile_pool(name="ps_m", bufs=1, space="PSUM"))

        # ---------------- constants / params ----------------
        ident = consts.tile([H, H], F32)
        make_identity(nc, ident)
        identh = consts.tile([128, 128], F16)
        make_identity(nc, identh)
        identf = consts.tile([128, 128], F32)
        make_identity(nc, identf)

        eps_sb = consts.tile([128, 1], F32)
        nc.vector.memset(eps_sb, LN_EPS)

        viota_i = consts.tile([V, 1], I32)
        nc.gpsimd.iota(viota_i, pattern=[[1, 1]], base=0, channel_multiplier=1)
        viota = consts.tile([V, 1], F32)
        nc.vector.tensor_copy(viota, viota_i)

        embed_sb = consts.tile([V, H], F32)
        w1_sb = consts.tile([H, 2 * H], F32)
        b1_sb = consts.tile([2 * H, 1], F32)
        w2_sb = consts.tile([2 * H, H], F32)
        rp_w_sb = consts.tile([H, H], F32)
        rp_b_sb = consts.tile([H, 1], F32)
        out_w_sb = consts.tile([H, V], F32)
        out_b_sb = consts.tile([V, 1], F32)
        for sb, p in (
            (embed_sb, embed_p), (w1_sb, w1_p), (b1_sb, b1_p), (w2_sb, w2_p),
            (rp_w_sb, rp_w_p), (rp_b_sb, rp_b_p),
            (out_w_sb, out_w_p), (out_b_sb, out_b_p),
        ):
            nc.sync.dma_start(out=sb, in_=p[:, :])

        b2B = consts.tile([128, H], F32)
        nc.sync.dma_start(
            out=b2B,
            in_=bass.AP(tensor=b2_p, offset=0, ap=[[0, 128], [1, H]]),
        )
        if not ln_trivial:
            g_bc = consts.tile([128, H], F32)
            bta_bc = consts.tile([128, H], F32)
            nc.sync.dma_start(
                out=g_bc, in_=bass.AP(tensor=ln_g_p, offset=0, ap=[[0, 128], [1, H]]))
            nc.sync.dma_start(
                out=bta_bc, in_=bass.AP(tensor=ln_b_p, offset=0, ap=[[0, 128], [1, H]]))

        seq_i = consts.tile([BS, L], I32)
        nc.sync.dma_start(out=seq_i, in_=seq_p[:, :])
        seq_f = consts.tile([BS, L], F32)
        nc.vector.tensor_copy(seq_f, seq_i)

        dummy_ps = ps_m.tile([128, 128], F32, tag="psS")
        nc.tensor.matmul(dummy_ps[0:BS, 0:BS], lhsT=ident[0:BS, 0:BS], rhs=ident[0:BS, 0:BS], start=True, stop=True)

        # transpose seq to [l, b] then scatter to phased DRAM staging:
        # seqT_sb[part p=16*ir+4*g+lrel, k, b] -> seqP[i=8k+ir, g*128+4b+lrel]
        seqT_sb = consts.tile([128, 4, BS], F32)
        for k in range(4):
            pst = ps_m.tile([128, 128], F32, tag="psS")
            nc.tensor.matmul(pst[:, 0:BS], lhsT=seq_f[:, 128 * k:128 * (k + 1)], rhs=ident[0:BS, 0:BS], start=True, stop=True)
            nc.vector.tensor_copy(seqT_sb[:, k, :], pst[:, 0:BS])
        # seqQ[b, k, 16ir+4g+lrel] = seqT_sb[16ir+4g+lrel, k, b]
        seqQ = consts.tile([BS, 4, 128], F32)
        for kq in range(4):
            psq = ps_m.tile([128, 128], F32, tag="psS")
            nc.tensor.matmul(psq[0:BS, :], lhsT=apx(seqT_sb[:, :, :], [[128, 128], [1, BS]], off=kq * BS), rhs=identf, start=True, stop=True)
            nc.vector.tensor_copy(seqQ[:, kq, :], psq[0:BS, :])
        # -> seqT_d[i=8k+ir, 128g + 4b + lrel]  (16B contiguous runs)
        for kq in range(4):
            for g in range(4):
                nc.sync.dma_start(
                    out=bass.AP(tensor=seqT_d,
                                offset=kq * 4096 + 128 * g,
                                ap=[[4, BS], [512, 8], [1, 4]]),
                    in_=apx(seqQ[:, :, :], [[512, BS], [16, 8], [1, 4]],
                            off=kq * 128 + 4 * g),
                )

        # embedT, w1p (fp32) then fp16 casts
        pse = ps_m.tile([128, 128], F32, tag="psS")
        nc.tensor.matmul(pse[0:H, 0:H], lhsT=embed_sb, rhs=ident, start=True, stop=True)
        embedT_sb = consts.tile([H, V], F32)
        nc.vector.tensor_copy(embedT_sb, pse[0:H, 0:H])
        psw = ps_m.tile([128, 128], F32, tag="psS")
        nc.tensor.matmul(psw[0:V, :], lhsT=embedT_sb, rhs=w1_sb, start=True, stop=True)
        w1p_sb = consts.tile([V, 2 * H], F32)
        nc.vector.tensor_copy(w1p_sb, psw[0:V, :])

        w1p_h = consts.tile([V, 2 * H], F16)
        embed_h = consts.tile([V, H], F16)
        w2_h = consts.tile([2 * H, H], F16)
        nc.scalar.activation(w1p_h, w1p_sb, AF.Copy)
        nc.scalar.activation(embed_h, embed_sb, AF.Copy)
        nc.scalar.activation(w2_h, w2_sb, AF.Copy)

        # ---------------- persistent buffers ----------------
        kn_ph = big.tile([128, 128, H], F16)     # phased kn  (pstride 8192)
        knd = big.tile([128, 16, L], F16)        # dense stacked kn^T (pstride 8192)
        zpzA = big.tile([128, 16, 128], F16)     # block lhsT (c01,s) (pstride 2048)
        zpzB = big.tile([128, 16, 128], F16)
        nc.gpsimd.memset(zpzA, 0.0)
        nc.gpsimd.memset(zpzB, 0.0)
        GdA = big.tile([BS, TB, TB], F16)        # in-block G (pstride 4096)
        GdB = big.tile([BS, TB, TB], F16)
        ktA = big.tile([H, 16, 128], F16)        # knTok (pstride 2048)
        ktB = big.tile([H, 16, 128], F16)
        GTd = big.tile([128, 16, TB], F16)       # staging (pstride 1024)

        n_sb = big.tile([BS, L], F32)            # sigma rows (pstride 512)
        s_all = big.tile([128, 128], F32)
        c_rep = big.tile([128, 128], F32)
        w_all = big.tile([128, 128], F32)
        y4 = big.tile([128, H], F32)
        nc.vector.memset(y4, 0.0)
        zpadf = big.tile([128, BS], F32)         # pair-major cols (pstride 32)
        nc.vector.memset(zpadf, 0.0)
        zpad16 = big.tile([128, BS], F16)
        nT16 = big.tile([TB, BS], F16)
        gmT16 = big.tile([TB, BS], F16)
        qv = big.tile([BS, H], F16)
        s511 = big.tile([BS, 1], F32)
        qvf = big.tile([BS, H], F32)
        qv16 = big.tile([BS, H], F16)
        qT = big.tile([H, BS], F32)

        zpz = (zpzA, zpzB)
        Gd = (GdA, GdB)
        kt = (ktA, ktB)

        def emit_tile(i):
            seqb = work.tile([V, TPT], F32)
            nc.sync.dma_start(
                out=seqb,
                in_=bass.AP(tensor=seqT_d, offset=i * 512, ap=[[0, V], [1, TPT]]),
            )
            oh = work.tile([V, TPT], F16)
            nc.vector.tensor_scalar(
                out=oh, in0=seqb, scalar1=viota[:, 0:1], scalar2=None,
                op0=OP.is_equal,
            )

            psB = ps_b.tile([2 * H, TPT], F32, tag="psB")
            nc.tensor.matmul(psB, lhsT=w1p_h, rhs=oh, start=True, stop=True)
            rT = work.tile([2 * H, TPT], F16)
            nc.scalar.activation(rT, psB, AF.Relu, bias=b1_sb[:, 0:1])

            psX = ps_x.tile([128, 4, H], F32, tag="psX")
            for g in range(4):
                nc.tensor.matmul(psX[:, g, :], lhsT=oh[:, 128 * g:128 * (g + 1)], rhs=embed_h, start=True, stop=False)
                nc.tensor.matmul(psX[:, g, :], lhsT=rT[:, 128 * g:128 * (g + 1)], rhs=w2_h, start=False, stop=True)

            x_sb = work.tile([128, 4, H], F32)
            nc.vector.tensor_tensor(
                x_sb, psX, apx(b2B[:, :], [[H, 128], [0, 4], [1, H]]), OP.add,
            )

            st = work.tile([128, 4, 6], F32)
            mv = work.tile([128, 4, 2], F32)
            for g in range(4):
                nc.vector.bn_stats(st[:, g, :], x_sb[:, g, :])
            for g in range(4):
                nc.vector.bn_aggr(mv[:, g, :], st[:, g, :])

            nrm = work.tile([128, 4, 1], F32)
            sstd = work.tile([128, 4, 1], F32)
            invn = work.tile([128, 4, 1], F32)
            rstd = work.tile([128, 4, 1], F32)
            var_ap = mv[:, :, 1:2]
            nc.scalar.activation(nrm, var_ap, AF.Sqrt, scale=float(H))
            nc.scalar.activation(sstd, var_ap, AF.Sqrt, bias=eps_sb[:, 0:1])
            nc.vector.reciprocal(invn, nrm)
            nc.vector.reciprocal(rstd, sstd)
            nc.vector.tensor_tensor(
                s_all[:, 4 * i:4 * (i + 1)], nrm[:, :, 0], rstd[:, :, 0], OP.mult,
            )

            kn_t = work.tile([128, 4, H], F32)
            if ln_trivial:
                for g in range(4):
                    nc.vector.tensor_scalar(
                        out=kn_t[:, g, :], in0=x_sb[:, g, :],
                        scalar1=mv[:, g, 0:1], scalar2=invn[:, g, :],
                        op0=OP.subtract, op1=OP.mult,
                    )
            else:
                h_t = work.tile([128, 4, H], F32)
                for g in range(4):
                    mu = mv[:, g, 0:1]
                    nc.vector.tensor_scalar(
                        out=h_t[:, g, :], in0=x_sb[:, g, :],
                        scalar1=mu, scalar2=rstd[:, g, :],
                        op0=OP.subtract, op1=OP.mult,
                    )
                    nc.vector.tensor_mul(h_t[:, g, :], h_t[:, g, :], g_bc)
                    nc.vector.tensor_add(h_t[:, g, :], h_t[:, g, :], bta_bc)
                ss = work.tile([128, 4, 1], F32)
                sn = work.tile([128, 4, 1], F32)
                rn = work.tile([128, 4, 1], F32)
                for g in range(4):
                    nc.vector.scalar_tensor_tensor(
                        out=kn_t[:, g, :], in0=h_t[:, g, :], scalar=1.0,
                        in1=h_t[:, g, :], op0=OP.mult, op1=OP.mult,
                        accum_out=ss[:, g, :],
                    )
                nc.scalar.activation(sn, ss, AF.Sqrt)
                nc.vector.tensor_scalar(sn, sn, 1e-12, None, op0=OP.max)
                nc.vector.reciprocal(rn, sn)
                for g in range(4):
                    nc.vector.tensor_scalar(
                        out=kn_t[:, g, :], in0=h_t[:, g, :],
                        scalar1=rn[:, g, :], scalar2=None, op0=OP.mult,
                    )
                nc.scalar.activation(s_all[:, 4 * i:4 * (i + 1)], sn[:, :, 0], AF.Copy)

            nc.scalar.activation(kn_ph[:, 4 * i:4 * (i + 1), :], kn_t, AF.Copy)

            # stacked fp16 transposes; upper batches land at psum parts 64:128
            psKn = ps_k.tile([128, 4, 128], F32, tag="psKn")
            for g in range(4):
                nc.tensor.matmul(
                    psKn[0:64, g, 0:64],
                    lhsT=kn_ph[0:64, 4 * i + g, :], rhs=identh[0:64, 0:64],
                    start=True, stop=True,
                )
                nc.tensor.matmul(
                    psKn[64:128, g, 64:128],
                    lhsT=kn_ph[64:128, 4 * i + g, :], rhs=identh[64:128, 64:128],
                    start=True, stop=True,
                )
            # evac into knd[(b01,h), p, t] and zpz_blk[(b01,h), p, c01*64+s]
            k = i // 4
            sib0 = (i % 4) * LT
            zt = zpz[k % 2]
            for half in range(2):
                # psKn pstride 512; free (g, col=4b+lrel): g:128, b:4, lrel:1
                src = apx(psKn[:, :, :], [[512, 64], [128, 4], [4, 16], [1, 4]],
                          off=half * (64 * 512 + 64))
                # knd pstride 8192; (p, t=16i+4g+lrel): b->512, g->4, lrel->1
                dk = apx(knd[:, :, :], [[8192, 64], [4, 4], [512, 16], [1, 4]],
                         off=half * 64 * 8192 + 16 * i)
                nc.scalar.activation(dk, src, AF.Copy)
                # zpz pstride 2048; (p, c01*64 + sib0+4g+lrel): b->128, g->4
                dz = apx(zt[:, :, :], [[2048, 64], [4, 4], [128, 16], [1, 4]],
                         off=half * (64 * 2048 + 64) + sib0)
                nc.scalar.activation(dz, src, AF.Copy)

        def emit_block_prep(k):
            zt = zpz[k % 2]
            ktk = kt[k % 2]
            gdk = Gd[k % 2]
            t0 = TB * k
            for p in range(16):
                psG = ps_g.tile([128, TB], F32, tag="psG")
                nc.tensor.matmul(
                    psG, lhsT=zt[:, p, :], rhs=knd[:, p, t0:t0 + TB],
                    start=True, stop=True,
                )
                nc.scalar.activation(GTd[:, p, :], psG, AF.Copy)
                psTk = ps_t.tile([TB, 128], F32, tag="psTk")
                nc.tensor.matmul(
                    psTk, lhsT=knd[:, p, t0:t0 + TB], rhs=identh,
                    start=True, stop=True,
                )
                nc.scalar.activation(ktk[:, p, :], psTk, AF.Copy)
                # Gd[2p+c01, j, i] <- GTd[64*c01+j, p, i]
                nc.sync.dma_start(
                    out=apx(gdk[:, :, :], [[4096, 2], [TB, TB], [1, TB]],
                            off=2 * p * 4096),
                    in_=apx(GTd[:, p, :], [[1024, 128], [1, TB]]),
                )

        def emit_chain_steps(k, jhi, jlo):
            gdk = Gd[k % 2]
            t0 = TB * k
            for j in range(jhi, jlo - 1, -1):
                s = t0 + j
                nc.vector.add_instruction(
                    mybir.InstTensorScalarPtr(
                        name=nc.vector.bass.get_next_instruction_name(),
                        is_scalar_tensor_tensor=True,
                        op0=OP.mult, op1=OP.subtract, reverse1=True,
                        ins=[
                            nc.vector.lower_ap(gdk[:, j, 0:j]),
                            nc.vector.lower_ap(n_sb[:, s:s + 1]),
                            nc.vector.lower_ap(n_sb[:, t0:t0 + j]),
                        ],
                        outs=[nc.vector.lower_ap(n_sb[:, t0:t0 + j])],
                    )
                )

        def emit_block_gm(k):
            t0 = TB * k
            nc.scalar.activation(zpad16, zpadf, AF.Copy)
            # gm oriented [t, sigma-col] (psum out base must be 0/32/64),
            # then PE-transposed back into n's [sigma, t] layout
            psGmT_f = ps_m.tile([128, 128], F32, tag="psS")
            psGmT = psGmT_f[0:TB, 0:BS]
            for p in range(16):
                nc.tensor.matmul(
                    psGmT[:, 2 * p:2 * p + 2],
                    lhsT=knd[:, p, t0:t0 + TB],
                    rhs=zpad16[:, 2 * p:2 * p + 2],
                    start=True, stop=True,
                )
            nc.scalar.activation(gmT16, psGmT, AF.Copy)
            psN_f = ps_m.tile([128, 128], F32, tag="psS")
            nc.tensor.matmul(psN_f[0:BS, 0:TB], lhsT=gmT16, rhs=identh[0:TB, 0:TB], start=True, stop=True)
            nc.vector.tensor_copy(n_sb[:, t0:t0 + TB], psN_f[0:BS, 0:TB])
            if k == NB - 1:
                nc.vector.memset(n_sb[:, L - 1:L], 0.0)

        def emit_block_tail(k):
            t0 = TB * k
            # y-accum for this block: c_rep (phased) <- n (sigma rows)
            for lrel in range(4):
                nc.sync.dma_start(
                    out=apx(c_rep[:, :], [[512, 16], [1, 16]],
                            off=lrel * 128 + 16 * k),
                    in_=apx(n_sb[:, :], [[1024, 16], [4, 16]], off=t0 + lrel),
                )
                nc.sync.dma_start(
                    out=apx(c_rep[:, :], [[512, 16], [1, 16]],
                            off=64 * 128 + lrel * 128 + 16 * k),
                    in_=apx(n_sb[:, :], [[1024, 16], [4, 16]], off=512 + t0 + lrel),
                )
            nc.vector.tensor_tensor(
                w_all[:, 16 * k:16 * (k + 1)],
                c_rep[:, 16 * k:16 * (k + 1)],
                s_all[:, 16 * k:16 * (k + 1)], OP.mult,
            )
            for ch in range(16 * k + 15, 16 * k - 1, -1):
                nc.vector.scalar_tensor_tensor(
                    out=y4, in0=kn_ph[:, ch, :], scalar=w_all[:, ch:ch + 1],
                    in1=y4, op0=OP.mult, op1=OP.add,
                )

            if k > 0:
                # z-update: z -= sum_{s in block} c_s kn_s
                psT_f = ps_m.tile([128, 128], F32, tag="psS")
                nc.tensor.matmul(psT_f[0:TB, 0:BS], lhsT=n_sb[:, t0:t0 + TB], rhs=ident[0:BS, 0:BS], start=True, stop=True)
                nc.scalar.activation(nT16, psT_f[0:TB, 0:BS], AF.Copy)
                psZ_f = ps_m.tile([128, 128], F32, tag="psS")
                psZ = psZ_f[:, 0:BS]
                ktk = kt[k % 2]
                for p in range(16):
                    nc.tensor.matmul(
                        psZ[:, 2 * p:2 * p + 2],
                        lhsT=ktk[:, p, :], rhs=nT16[:, 2 * p:2 * p + 2],
                        start=True, stop=True,
                    )
                nc.vector.tensor_tensor(
                    apx(zpadf[:, :], [[32, 64], [2, 16]]),
                    apx(zpadf[:, :], [[32, 64], [2, 16]]),
                    apx(psZ_f[:, :], [[128, 64], [2, 16]]),
                    OP.subtract,
                )
                nc.vector.tensor_tensor(
                    apx(zpadf[:, :], [[32, 64], [2, 16]], off=64 * 32 + 1),
                    apx(zpadf[:, :], [[32, 64], [2, 16]], off=64 * 32 + 1),
                    apx(psZ_f[:, :], [[128, 64], [2, 16]], off=64 * 128 + 1),
                    OP.subtract,
                )

        # ---------------- main pipeline, reverse tile order ----------------
        pending = None
        for i in range(NT - 1, -1, -1):
            emit_tile(i)
            if i % 4 != 0 and pending is not None:
                k, jhi = pending
                jlo = max(jhi - 19, 1)
                emit_chain_steps(k, jhi, jlo)
                if jlo == 1:
                    emit_block_tail(k)
                    pending = None
                else:
                    pending = (k, jlo - 1)
            if i % 4 == 0:
                if pending is not None:
                    k, jhi = pending
                    emit_chain_steps(k, jhi, 1)
                    emit_block_tail(k)
                    pending = None
                k = i // 4
                emit_block_prep(k)
                if k == NB - 1:
                    # q = s_511 * kn_511; seed zpad (pair-major, b01-padded)
                    nc.sync.dma_start(
                        out=qv,
                        in_=apx(kn_ph[:, :, :], [[4 * 8192, BS], [1, H]],
                                off=3 * 8192 + 127 * H),
                    )
                    nc.sync.dma_start(
                        out=s511,
                        in_=apx(s_all[:, :], [[4 * 128, BS], [1, 1]],
                                off=3 * 128 + 127),
                    )
                    nc.vector.tensor_scalar(
                        out=qvf, in0=qv, scalar1=s511[:, 0:1], scalar2=None, op0=OP.mult,
                    )
                    nc.scalar.activation(qv16, qvf, AF.Copy)
                    psQ_f = ps_m.tile([128, 128], F32, tag="psS")
                    nc.tensor.matmul(psQ_f[0:H, 0:BS], lhsT=qv16, rhs=identh[0:BS, 0:BS], start=True, stop=True)
                    nc.scalar.activation(qT, psQ_f[0:H, 0:BS], AF.Copy)
                    nc.vector.tensor_copy(
                        apx(zpadf[:, :], [[32, 64], [2, 16]]),
                        qT[0:64, 0:16],
                    )
                    nc.sync.dma_start(
                        out=apx(zpadf[:, :], [[32, 64], [2, 16]], off=64 * 32 + 1),
                        in_=qT[0:64, 16:BS],
                    )
                emit_block_gm(k)
                pending = (k, TB - 1)
        if pending is not None:
            k, jhi = pending
            emit_chain_steps(k, jhi, 1)
            emit_block_tail(k)
            pending = None

        if os.environ.get("KDBG", "0") == "1":
            nc.sync.dma_start(out=dbg_knph[:, :], in_=apx(kn_ph[:, :, :], [[8192, 128], [1, 128 * H]]))
            nc.sync.dma_start(out=dbg_knd[:, :], in_=apx(knd[:, :, :], [[8192, 128], [1, 16 * L]]))
            nc.sync.dma_start(out=dbg_n[:, :], in_=n_sb)
            nc.sync.dma_start(out=dbg_gd7[:, :], in_=apx(Gd[(NB - 1) % 2][:, :, :], [[4096, BS], [1, TB * TB]]))
            nc.sync.dma_start(out=dbg_zp[:, :], in_=zpadf)
            nc.sync.dma_start(out=dbg_sall[:, :], in_=s_all)

        # ---------------- tail: y reduce + projections ----------------
        yp = [big.tile([BS, H], F32, name=f"yp{_l}") for _l in range(4)]
        for lrel in range(4):
            nc.sync.dma_start(
                out=yp[lrel],
                in_=apx(y4[:, :], [[4 * H, BS], [1, H]], off=lrel * H),
            )
        y_sb = big.tile([BS, H], F32)
        nc.vector.tensor_add(y_sb, yp[0], yp[1])
        nc.vector.tensor_add(y_sb, y_sb, yp[2])
        nc.vector.tensor_add(y_sb, y_sb, yp[3])

        psF = ps_m.tile([128, 128], F32, tag="psS")
        nc.tensor.matmul(psF[0:H, 0:BS], lhsT=y_sb, rhs=ident[0:BS, 0:BS], start=True, stop=True)
        yT = big.tile([H, BS], F32)
        nc.vector.tensor_copy(yT, psF[0:H, 0:BS])

        psG2 = ps_m.tile([128, 128], F32, tag="psS")
        nc.tensor.matmul(psG2[0:H, 0:BS], lhsT=rp_w_sb, rhs=yT, start=True, stop=True)
        r1 = big.tile([H, BS], F32)
        nc.scalar.activation(r1, psG2[0:H, 0:BS], AF.Identity, bias=rp_b_sb[:, 0:1])

        psH = ps_m.tile([128, 128], F32, tag="psS")
        nc.tensor.matmul(psH[0:V, 0:BS], lhsT=out_w_sb, rhs=r1, start=True, stop=True)
        r2 = big.tile([V, BS], F32)
        nc.scalar.activation(r2, psH[0:V, 0:BS], AF.Identity, bias=out_b_sb[:, 0:1])

        psI = ps_m.tile([128, 128], F32, tag="psS")
        nc.tensor.matmul(psI[0:BS, 0:V], lhsT=r2, rhs=ident, start=True, stop=True)
        o_sb = big.tile([BS, V], F32)
        nc.vector.tensor_copy(o_sb, psI[0:BS, 0:V])
        nc.sync.dma_start(out=out_p[:, :], in_=o_sb)

    nc.finalize()
    return nc


_CACHE = {}


def _run(inputs, trace=False, **kw):
    seq = np.asarray(inputs["seq"]).astype(np.int32)
    embed = np.asarray(inputs["embed"], np.float32)
    w1 = np.asarray(inputs["w1"], np.float32)
    b1 = np.asarray(inputs["b1"], np.float32).reshape(2 * H, 1)
    w2 = np.asarray(inputs["w2"], np.float32)
    b2 = np.asarray(inputs["b2"], np.float32).reshape(H, 1)
    ln_g = np.asarray(inputs["ln_g"], np.float32).reshape(1, H)
    ln_b = np.asarray(inputs["ln_b"], np.float32).reshape(1, H)
    rp_w = np.asarray(inputs["rp_w"], np.float32)
    rp_b = np.asarray(inputs["rp_b"], np.float32).reshape(H, 1)
    out_w = np.asarray(inputs["out_w"], np.float32)
    out_b = np.asarray(inputs["out_b"], np.float32).reshape(V, 1)

    ln_trivial = bool(np.all(ln_g == 1.0) and np.all(ln_b == 0.0))
    if ln_trivial not in _CACHE:
        _CACHE[ln_trivial] = build_program(ln_trivial)
    nc = _CACHE[ln_trivial]

    in_maps = []
    for c in range(NCORES):
        in_maps.append({
            "seq": seq[BS * c:BS * (c + 1)],
            "embed": embed, "w1": w1, "b1": b1, "w2": w2, "b2": b2,
            "ln_g": ln_g, "ln_b": ln_b,
            "rp_w": rp_w, "rp_b": rp_b, "out_w": out_w, "out_b": out_b,
        })
    br = run_bass_kernel_spmd(nc, in_maps, list(range(NCORES)), trace=trace, **kw)
    out = np.concatenate([r["out"] for r in br.results], axis=0)
    return out, br


def kernel(**inputs) -> np.ndarray:
    return _run(inputs)[0]
